# revision 9
# baseline (speedup 1.0000x reference)
"""Trainium2 Bass kernel for nn_DebugQuantizedLinear.

Computes out = x @ W_deq.T where
  W_deq = ((W_q - zeros) * scales).reshape(K, N) * mu2[:, None] * mu1[None, :]
  x: [B, N] f32, W_q: [K, N] int32 (values 0..15), out: [B, K] f32
  K=11008, N=4096, B=8192, group size 64 along N (NG=64 groups).

Strategy (8 NeuronCores, tensor-parallel along K):
  - K padded 11008 -> 11264 = 8 * 1408; core c owns rows [c*1408, (c+1)*1408).
  - Host re-encodes the quantized weights with the zero-point folded in:
      Q8 = 8*W_q - round(8*zeros)  (int8, range +-120)
      S  = scales * mu2 / 8        (bf16 table, expanded to [N, kc] by repeat)
    so W_deq.T = (Q8 * mu1[n]) * S_rep up to the zero-point rounding
    (adds ~5e-3 relative error vs the 2e-2 budget). x is transposed and
    cast fp16 host-side.
  - Device dequant is ONE fused DVE op per [128, 1408] n-tile slab
    (scalar_tensor_tensor: (Q8 * mu1[p]) * S -> fp16), written straight
    into the SBUF-resident transposed weights wdqT (32 slabs, 11.5 MB).
    No PE transposes, no ACT stage, minimal producer DMA (wq int8 on the
    SP queue, S bf16 on the ACT queue).
  - Matmuls start with slab 0: the first 8 output tiles (h=0, kt 0..7)
    accumulate slab-by-slab across all 8 PSUM banks, riding the producer
    at ~full PE duty; everything after is a pure back-to-back stream
    (5632 MMs total, FD=512, LDWEIGHTS hidden by the PE reorder window).
  - Output tiles drain PSUM->SBUF as fp16 (ACT) and DMA to DRAM
    outT [kc, B] fp16; host assembles out[B, K] f32.

HBM per core ~104 MB (vs ~197 baseline), which also avoids the chip
power-throttle (K=13/16 downclock) the baseline suffered for ~60% of
its runtime.
"""

import os
from contextlib import ExitStack

import numpy as np

K, N, B = 11008, 4096, 8192
GROUP = 64
NG = N // GROUP
NCORES = 8
KC = 1408               # per-core padded K rows
KPAD = KC * NCORES      # 11264
P = 128

_PROGRAM_CACHE = {}
LAST_RESULTS = None     # BassKernelResults of the most recent run (for test.py)


def _build_program(kc=KC, b=B, bh=512):
    """Build the SPMD Bass program (identical on all cores)."""
    import concourse.bacc as bacc
    import concourse.mybir as mybir
    from concourse.tile import TileContext

    f32 = mybir.dt.float32
    f16 = mybir.dt.float16
    bf = mybir.dt.bfloat16
    i8 = mybir.dt.int8
    mult = mybir.AluOpType.mult

    nkt = kc // P           # 11 k-tiles per core
    nnt = N // P            # 32 n-tiles
    nh = b // bh            # 16 half-panels
    NA = 8                  # out-tiles riding the producer (PSUM banks)

    nc = bacc.Bacc(num_swdge_queues=4)
    xT = nc.declare_dram_parameter("xT", [N, b], f16, isOutput=False)
    wq = nc.declare_dram_parameter("wq", [N, kc], i8, isOutput=False)
    srep = nc.declare_dram_parameter("srep", [N, kc], bf, isOutput=False)
    mu1 = nc.declare_dram_parameter("mu1", [P, nnt], f32, isOutput=False)
    outT = nc.declare_dram_parameter("outT", [kc, b], f16, isOutput=True)

    with TileContext(nc) as tc, ExitStack() as ctx:
        const = ctx.enter_context(tc.tile_pool(name="const", bufs=1))
        mu1_t = const.tile([P, nnt], f32, name="mu1_t")
        nc.gpsimd.dma_start(out=mu1_t[:, :], in_=mu1[:, :])

        # SBUF-resident transposed dequantized weights: per n-tile
        # [128 n-partitions, kc] fp16.
        wdqT = [const.tile([P, kc], f16, name=f"wdqT_{nt}") for nt in range(nnt)]

        wqpool = ctx.enter_context(tc.tile_pool(name="wqpool", bufs=4))
        spool = ctx.enter_context(tc.tile_pool(name="spool", bufs=4))
        xpool = ctx.enter_context(tc.tile_pool(name="xpool", bufs=2))
        opsum = ctx.enter_context(tc.tile_pool(name="opsum", bufs=8, space="PSUM"))
        opool = ctx.enter_context(tc.tile_pool(name="opool", bufs=3))
        pstage = ctx.enter_context(tc.tile_pool(name="pstage", bufs=NA))

        def x_src(h):
            return xT[:, h * bh:(h + 1) * bh].rearrange("(t p) b -> p t b", p=P)

        def load_x_chunk(xh, h, q):
            sl = slice(q * (nnt // 4), (q + 1) * (nnt // 4))
            nc.sync.dma_start(out=xh[:, sl, :], in_=x_src(h)[:, sl, :])

        def producer(nt):
            """Dequantize n-tile slab nt into wdqT[nt]: one fused DVE op.

            Slab DMA is split across the SP and ACT HW-DGE rings so each
            ring stays under the PE's 1.73us/slab consumption rate."""
            wq_t = wqpool.tile([P, kc], i8, name="wq_t")
            nc.sync.dma_start(out=wq_t[:, :], in_=wq[nt * P:(nt + 1) * P, :])
            s_t = spool.tile([P, kc], bf, name="s_t")
            nc.sync.dma_start(
                out=s_t[0:44, :], in_=srep[nt * P:nt * P + 44, :])
            nc.scalar.dma_start(
                out=s_t[44:P, :], in_=srep[nt * P + 44:(nt + 1) * P, :])
            nc.vector.scalar_tensor_tensor(
                out=wdqT[nt][:, :], in0=wq_t[:, :],
                scalar=mu1_t[:, nt:nt + 1], in1=s_t[:, :],
                op0=mult, op1=mult)

        def drain(ps, h, kt):
            ot = opool.tile([P, bh], f16, name="ot")
            nc.scalar.copy(ot[:, :], ps[:, :])
            nc.scalar.dma_start(
                out=outT[kt * P:(kt + 1) * P, h * bh:(h + 1) * bh], in_=ot[:, :])

        def full_tile(h, kt, xh):
            ps = opsum.tile([P, bh], f32, name="ops")
            for nt in range(nnt):
                nc.tensor.matmul(
                    ps[:, :],
                    lhsT=wdqT[nt][:, kt * P:(kt + 1) * P],
                    rhs=xh[:, nt, :],
                    start=(nt == 0), stop=(nt == nnt - 1))
            drain(ps, h, kt)

        # HAM warm-up: ~4.2us of tiny FD=64 dummy matmuls so the PE clock
        # gate opens (1.2 -> 2.4 GHz) before the first real matmul lands.
        warm = const.tile([P, 64], f16, name="warm")
        nc.vector.memset(warm[:, :], 0.0)
        wps = opsum.tile([64, 64], f32, name="ops")
        for _ in range(72):
            nc.tensor.matmul(wps[:, :], lhsT=warm[:, :64], rhs=warm[:, :],
                             start=True, stop=True)
        wsc = opool.tile([64, 64], f16, name="ot")
        nc.scalar.copy(wsc[:, :], wps[:, :])

        # h=0 panel, split accumulation so the PE stays busy for the whole
        # producer window:
        #   slabs 0..15:  tiles (kt0-7, h0) accumulate part1 -> staged fp16
        #   slabs 16..31: 8 fresh tiles (kt8-10 h0, kt0-4 h1) do the
        #                 current slab AND catch up one early slab each
        #                 step (16 MMs/slab), using banks freed by part1.
        #   after:        (kt0-7, h0) part2 over slabs 16..31 + staged add.
        xh = xpool.tile([P, nnt, bh], f16, name="xh")
        nc.gpsimd.dma_start(out=xh[:, 0:1, :], in_=x_src(0)[:, 0:1, :])
        producer(0)
        HALF = nnt // 2
        tileB = [(kt, 0) for kt in range(NA, nkt)] \
            + [(kt, 1) for kt in range(NA - (nkt - NA))]
        psA = [opsum.tile([P, bh], f32, name="ops") for kt in range(NA)]
        for nt in range(HALF):
            if nt + 1 < nnt:
                nc.gpsimd.dma_start(
                    out=xh[:, nt + 1:nt + 2, :], in_=x_src(0)[:, nt + 1:nt + 2, :])
            if nt > 0:
                producer(nt)
            for kt in range(NA):
                nc.tensor.matmul(
                    psA[kt][:, :],
                    lhsT=wdqT[nt][:, kt * P:(kt + 1) * P],
                    rhs=xh[:, nt, :],
                    start=(nt == 0), stop=(nt == HALF - 1),
                    skip_group_check=True)
        stage = []
        for kt in range(NA):
            st = pstage.tile([P, bh], f16, name="st")
            nc.scalar.copy(st[:, :], psA[kt][:, :])
            stage.append(st)
        xh1 = xpool.tile([P, nnt, bh], f16, name="xh")
        psB = [opsum.tile([P, bh], f32, name="ops") for _ in range(NA)]

        def xof(h):
            return xh if h == 0 else xh1

        for nt in range(HALF, nnt):
            nc.gpsimd.dma_start(
                out=xh1[:, nt:nt + 1, :], in_=x_src(1)[:, nt:nt + 1, :])
            nc.scalar.dma_start(
                out=xh1[:, nt - HALF:nt - HALF + 1, :],
                in_=x_src(1)[:, nt - HALF:nt - HALF + 1, :])
            if nt + 1 < nnt:
                nc.gpsimd.dma_start(
                    out=xh[:, nt + 1:nt + 2, :], in_=x_src(0)[:, nt + 1:nt + 2, :])
            producer(nt)
            for j, (kt, h) in enumerate(tileB):
                nc.tensor.matmul(
                    psB[j][:, :],
                    lhsT=wdqT[nt][:, kt * P:(kt + 1) * P],
                    rhs=xof(h)[:, nt, :],
                    start=(nt == HALF), stop=False,
                    skip_group_check=True)
            for j, (kt, h) in enumerate(tileB):
                nc.tensor.matmul(
                    psB[j][:, :],
                    lhsT=wdqT[nt - HALF][:, kt * P:(kt + 1) * P],
                    rhs=xof(h)[:, nt - HALF, :],
                    start=False, stop=(nt == nnt - 1),
                    skip_group_check=True)
        for j, (kt, h) in enumerate(tileB):
            drain(psB[j], h, kt)
        # part2 for (kt0-7, h0): slabs 16..31 into fresh banks, then add
        # the staged part1 during the fp16 drain.
        for kt in range(NA):
            ps = opsum.tile([P, bh], f32, name="ops")
            for nt in range(HALF, nnt):
                nc.tensor.matmul(
                    ps[:, :],
                    lhsT=wdqT[nt][:, kt * P:(kt + 1) * P],
                    rhs=xh[:, nt, :],
                    start=(nt == HALF), stop=(nt == nnt - 1))
            ot = opool.tile([P, bh], f16, name="ot")
            nc.vector.tensor_add(ot[:, :], ps[:, :], stage[kt][:, :])
            nc.scalar.dma_start(
                out=outT[kt * P:(kt + 1) * P, 0:bh], in_=ot[:, :])
        # rest of h=1 (tiles not covered by tileB), then h=2..15.
        done_h1 = {kt for kt, h in tileB if h == 1}
        for kt in range(nkt):
            if kt not in done_h1:
                full_tile(1, kt, xh1)
        for h in range(2, nh):
            xh2 = xpool.tile([P, nnt, bh], f16, name="xh")
            for q in range(4):
                load_x_chunk(xh2, h, q)
            for kt in range(nkt):
                full_tile(h, kt, xh2)

    nc.finalize()
    return nc


def _get_program(key=()):
    if key not in _PROGRAM_CACHE:
        _PROGRAM_CACHE[key] = _build_program(*key) if key else _build_program()
    return _PROGRAM_CACHE[key]


def kernel(x, W_q, zeros, scales, mu1, mu2):
    global LAST_RESULTS
    import ml_dtypes
    from concourse.bass_utils import run_bass_kernel_spmd

    x = np.asarray(x)
    W_q = np.asarray(W_q)
    zeros = np.asarray(zeros)
    scales = np.asarray(scales)
    mu1 = np.asarray(mu1)
    mu2 = np.asarray(mu2)

    # Host-side prep: transposes/casts/repeats, zero-point folding into
    # the int8 encoding, and combining the small [K, NG] scale tables.
    xT16 = np.ascontiguousarray(x.T).astype(np.float16)        # [N, B]
    zi = np.rint(zeros[:, :, 0] * 8.0).astype(np.int16)        # [K, NG]
    q8 = (8 * W_q.astype(np.int16)
          - np.repeat(zi, GROUP, axis=1)).astype(np.int8)      # [K, N]
    wqT = np.zeros((N, KPAD), dtype=np.int8)
    wqT[:, :K] = q8.T
    sc = np.zeros((KPAD, NG), dtype=np.float32)
    sc[:K] = scales[:, :, 0] * mu2[:, None] * 0.125
    mu1r = np.ascontiguousarray(mu1.reshape(N // P, P).T)      # [128, 32] f32

    in_maps = []
    for c in range(NCORES):
        lo, hi = c * KC, (c + 1) * KC
        in_maps.append({
            "xT": xT16,
            "wq": np.ascontiguousarray(wqT[:, lo:hi]),
            "srep": np.ascontiguousarray(
                np.repeat(sc[lo:hi].T.astype(ml_dtypes.bfloat16), GROUP, axis=0)),
            "mu1": mu1r,
        })

    nc = _get_program()
    trace = bool(os.environ.get("KERNEL_TRACE"))
    res = run_bass_kernel_spmd(nc, in_maps, list(range(NCORES)), trace=trace)
    LAST_RESULTS = res

    out = np.empty((B, K), dtype=np.float32)
    for c in range(NCORES):
        lo = c * KC
        hi = min(lo + KC, K)
        out[:, lo:hi] = res.results[c]["outT"][:hi - lo].T.astype(np.float32)
    return out
